# revision 1
# baseline (speedup 1.0000x reference)
"""MoE (single shared expert) kernel for 8 trn2 NeuronCores.

Math: the reference's top-2 gating over 64 "experts" feeds a single shared
FFN, and the renormalized top-2 weights sum to s/(s+1e-9) with s >= 1/64,
i.e. 1 up to <= 6.4e-8 relative -- below f32 rounding noise.  The whole
module therefore reduces to:  out = silu(x @ up_w.T) @ down_w.T.

Sharding (8 cores): 2D = 4 token-groups x 2 expert-halves.
Each core (tg, eg) computes the partial
    ytp = ( silu(X[tg] @ up_w[eg].T) @ down_w[:, eg].T ).T      [D, TC]
with X[tg] = 2048 tokens, eg = half of the 2048 expert dims.  The host
sums the two partials of each token group and transposes back.

All device matmuls run with float32r operand APs (full f32 storage,
single-pass PE streaming) by default; MOE_MM_DTYPE=f32 selects exact fp32.
"""

import os
import sys

import numpy as np

for _p in ("/opt/trn_rl_repo",):
    if os.path.isdir(_p) and _p not in sys.path:
        sys.path.insert(0, _p)

import concourse.bass as bass
import concourse.mybir as mybir
import concourse.tile as tile

F32 = mybir.dt.float32
F32R = mybir.dt.float32r


def _ensure_axon_hooks_shim():
    """bass_utils' trace path imports antenv.axon_hooks, which this image
    lacks; give it a no-op hook module so BASS_TRACE=1 degrades gracefully."""
    import types
    if "antenv.axon_hooks" in sys.modules:
        return
    try:
        import antenv
    except ImportError:
        return
    if hasattr(antenv, "axon_hooks"):
        return
    ah = types.ModuleType("antenv.axon_hooks")
    ah._hook = None
    ah.set_axon_ntff_profile_hook = lambda h: setattr(ah, "_hook", h)
    ah.get_axon_ntff_profile_hook = lambda: ah._hook
    sys.modules["antenv.axon_hooks"] = ah
    antenv.axon_hooks = ah


_ensure_axon_hooks_shim()


def _split_multi_waits(nc):
    """This container's walrus encodes at most ONE sync wait per engine
    instruction ("Too many sync wait commands").  Tile routinely emits
    instructions waiting on several semaphores; hoist the extra waits onto
    single-wait NoOps inserted just before, on the same engine."""
    n = 0
    for f in nc.m.functions:
        for blk in f.blocks:
            insts = blk.instructions
            out = []
            for inst in insts:
                si = inst.sync_info
                waits = list(si.on_wait) if si and si.on_wait else []
                if len(waits) > 1:
                    for w in waits[:-1]:
                        n += 1
                        nop = mybir.InstNoOp(name=f"I-wsplit-{n}", ins=[], outs=[])
                        nop.engine = inst.engine
                        nop.sync_info = mybir.SyncInfo(on_wait=[w], on_update=[])
                        nc.register_instruction(nop)
                        out.append(nop)
                    si.on_wait = [waits[-1]]
                out.append(inst)
            if n:
                insts[:] = out
    return n

# Problem shape (hardcoded per contract)
B, S, D, ED = 4, 2048, 1024, 2048
T = B * S                    # 8192 tokens
TG, EG = 4, 2                # token groups x expert-half groups = 8 cores
TC = T // TG                 # tokens per core      = 2048
EC = ED // EG                # expert dims per core = 1024
TT = 512                     # token tile (matmul free dim)
NTT = TC // TT               # 4 token tiles
NDT = D // 128               # 8 d-tiles (contraction 1 / output rows)
NET = EC // 128              # 8 e-tiles (output rows 1 / contraction 2)

_CACHE = {}
LAST_RESULTS = None          # BassKernelResults of the most recent run


def build_nc(mode: str = "f32r") -> bass.Bass:
    """One-core SPMD program: ytp[D, TC] = (silu(x @ upT) @ dwnT).T partial."""
    mm_dt = {"f32r": F32R, "f32": F32}[mode]

    nc = bass.Bass()
    xt = nc.dram_tensor("xt", [D, TC], mm_dt, kind="ExternalInput")
    upw = nc.dram_tensor("upw", [D, EC], mm_dt, kind="ExternalInput")
    dwn = nc.dram_tensor("dwn", [EC, D], mm_dt, kind="ExternalInput")
    ytp = nc.dram_tensor("ytp", [D, TC], F32, kind="ExternalOutput")

    with tile.TileContext(nc) as tc:
        with (
            tc.tile_pool(name="wpool", bufs=1) as wpool,
            tc.tile_pool(name="xpool", bufs=16) as xpool,
            tc.tile_pool(name="hpool", bufs=16) as hpool,
            tc.tile_pool(name="ypool", bufs=6) as ypool,
            tc.tile_pool(name="psum", bufs=8, space="PSUM") as psum,
        ):
            # Whole-tile weight DMAs (contiguous 512KB, full DMA efficiency),
            # emitted in consumption order:
            #   up[0..3] -> x(tt0) -> up[4..7] -> x(tt1) -> dn -> x(tt2) ...
            # tt0's up-projection runs as two half-K sweeps so the PE starts
            # after only ~3MB.  PE program order is software-pipelined
            #   L1(0) L1(1) L2(0) L1(2) L2(1) L1(3) L2(2) L2(3)
            # so dn's 8MB stream hides behind L1(1); L2(0) consumes dn
            # ei-major (8 PSUM banks) to avoid waiting for the whole tensor.
            up_sb = [wpool.tile([128, EC], mm_dt, tag=f"up{di}", name=f"up{di}")
                     for di in range(NDT)]
            dn_sb = [wpool.tile([128, D], mm_dt, tag=f"dn{ei}", name=f"dn{ei}")
                     for ei in range(NET)]

            def load_up(dis):
                for di in dis:
                    nc.sync.dma_start(
                        out=up_sb[di][:], in_=upw[di * 128:(di + 1) * 128, :]
                    )

            def load_x(tt):
                t0 = tt * TT
                xs = []
                for di in range(NDT):
                    xtile = xpool.tile([128, TT], mm_dt, tag="x")
                    nc.sync.dma_start(
                        out=xtile[:],
                        in_=xt[di * 128:(di + 1) * 128, t0:t0 + TT],
                    )
                    xs.append(xtile)
                return xs

            # Warm the PE (HAM clock gate) with dummy matmuls on memset
            # tiles while the initial DMAs stream: the 128x128 array starts
            # at 1.2GHz and only reaches 2.4GHz after ~3.4us of sustained
            # work.  These have no DMA dependency, so they fill the
            # DMA-ramp window; the copy at the end keeps them from DCE.
            n_warm = int(os.environ.get("MOE_WARM_MM", "0"))
            if n_warm:
                wz = wpool.tile([128, 128], F32, tag="warmw")
                xz = xpool.tile([128, TT], F32, tag="warmx", bufs=1)
                nc.vector.memset(wz[:], 0.0)
                nc.vector.memset(xz[:], 0.0)
                wps = psum.tile([128, TT], F32, tag="ps", name="warm_ps")
                for _ in range(n_warm):
                    nc.tensor.matmul(wps[:], wz[:], xz[:], start=True, stop=True)
                wsink = ypool.tile([128, TT], F32, tag="y", name="warm_sink")
                nc.vector.tensor_copy(wsink[:], wps[:])

            # pairs (up[di], x0[di]) so MM(eb0, di) unblocks after ~0.75MB*di.
            # (Splitting these finer starts the PE earlier but into a sparse
            # DMA-paced stream, which makes the HAM clock-gate oscillate and
            # costs more than it saves -- measured.)
            xs_all = {0: []}
            for di in range(NDT):
                load_up([di])
                xtile = xpool.tile([128, TT], mm_dt, tag="x", name=f"x0_{di}")
                nc.sync.dma_start(out=xtile[:], in_=xt[di * 128:(di + 1) * 128, 0:TT])
                xs_all[0].append(xtile)
            xs_all[1] = load_x(1)
            for ei in range(NET):
                nc.sync.dma_start(
                    out=dn_sb[ei][:], in_=dwn[ei * 128:(ei + 1) * 128, :]
                )

            hs_all = {}

            def loop1(tt, split_k):
                xs = xs_all[tt] if tt in xs_all else load_x(tt)
                pss = []
                if split_k:
                    pss = [psum.tile([128, TT], F32, tag="ps", name=f"ps1_{tt}_{eb}")
                           for eb in range(NET)]
                    for dis in (range(0, NDT // 2), range(NDT // 2, NDT)):
                        for eb in range(NET):
                            for di in dis:
                                nc.tensor.matmul(
                                    pss[eb][:],
                                    up_sb[di][:, eb * 128:(eb + 1) * 128],
                                    xs[di][:],
                                    start=(di == 0),
                                    stop=(di == NDT - 1),
                                )
                hs = []
                for eb in range(NET):
                    if split_k:
                        ps = pss[eb]
                    else:
                        ps = psum.tile([128, TT], F32, tag="ps",
                                       name=f"ps1_{tt}_{eb}")
                        for di in range(NDT):
                            nc.tensor.matmul(
                                ps[:],
                                up_sb[di][:, eb * 128:(eb + 1) * 128],
                                xs[di][:],
                                start=(di == 0),
                                stop=(di == NDT - 1),
                            )
                    sg = hpool.tile([128, TT], F32, tag="sg", bufs=3)
                    nc.scalar.activation(
                        sg[:], ps[:], mybir.ActivationFunctionType.Sigmoid
                    )
                    h = hpool.tile([128, TT], mm_dt, tag="h", bufs=20)
                    nc.vector.tensor_mul(h[:], ps[:], sg[:])
                    hs.append(h)
                hs_all[tt] = hs

            def loop2(tt, ei_major):
                t0 = tt * TT
                hs = hs_all.pop(tt)
                if ei_major:
                    ps2s = [psum.tile([128, TT], F32, tag="ps", name=f"ps2_{tt}_{db}")
                            for db in range(NDT)]
                    for ei in range(NET):
                        for db in range(NDT):
                            nc.tensor.matmul(
                                ps2s[db][:],
                                dn_sb[ei][:, db * 128:(db + 1) * 128],
                                hs[ei][:],
                                start=(ei == 0),
                                stop=(ei == NET - 1),
                            )
                    for db in range(NDT):
                        y = ypool.tile([128, TT], F32, tag="y")
                        nc.vector.tensor_copy(y[:], ps2s[db][:])
                        nc.sync.dma_start(
                            out=ytp[db * 128:(db + 1) * 128, t0:t0 + TT],
                            in_=y[:],
                        )
                else:
                    for db in range(NDT):
                        if tt == NTT - 1 and db == NDT - 1:
                            # Last group of the kernel: split into column
                            # halves so the first half's copy+DMA overlap the
                            # second half's matmuls, shortening the tail chain.
                            dsl = slice(db * 128, (db + 1) * 128)
                            half = TT // 2
                            psA = psum.tile([128, half], F32, tag="ps",
                                            name="ps2_last_a")
                            psB = psum.tile([128, half], F32, tag="ps",
                                            name="ps2_last_b")
                            for ei in range(NET):
                                nc.tensor.matmul(
                                    psA[:], dn_sb[ei][:, dsl],
                                    hs[ei][:, 0:half],
                                    start=(ei == 0), stop=(ei == NET - 1),
                                )
                            yA = ypool.tile([128, half], F32, tag="y2", bufs=2)
                            nc.vector.tensor_copy(yA[:], psA[:])
                            nc.sync.dma_start(
                                out=ytp[dsl, t0:t0 + half], in_=yA[:],
                            )
                            for ei in range(NET):
                                nc.tensor.matmul(
                                    psB[:], dn_sb[ei][:, dsl],
                                    hs[ei][:, half:TT],
                                    start=(ei == 0), stop=(ei == NET - 1),
                                )
                            yB = ypool.tile([128, half], F32, tag="y2", bufs=2)
                            nc.vector.tensor_copy(yB[:], psB[:])
                            nc.sync.dma_start(
                                out=ytp[dsl, t0 + half:t0 + TT], in_=yB[:],
                            )
                            continue
                        ps2 = psum.tile([128, TT], F32, tag="ps",
                                        name=f"ps2_{tt}_{db}")
                        for ei in range(NET):
                            nc.tensor.matmul(
                                ps2[:],
                                dn_sb[ei][:, db * 128:(db + 1) * 128],
                                hs[ei][:],
                                start=(ei == 0),
                                stop=(ei == NET - 1),
                            )
                        y = ypool.tile([128, TT], F32, tag="y")
                        nc.vector.tensor_copy(y[:], ps2[:])
                        nc.sync.dma_start(
                            out=ytp[db * 128:(db + 1) * 128, t0:t0 + TT],
                            in_=y[:],
                        )

            loop1(0, split_k=True)
            loop1(1, split_k=False)
            loop2(0, ei_major=True)
            loop1(2, split_k=False)
            loop2(1, ei_major=False)
            loop1(3, split_k=False)
            loop2(2, ei_major=False)
            loop2(3, ei_major=False)

    _split_multi_waits(nc)
    nc.finalize()
    return nc


def _get_nc(mode: str) -> bass.Bass:
    if mode not in _CACHE:
        _CACHE[mode] = build_nc(mode)
    return _CACHE[mode]


def kernel(x, gate_w, up_w, down_w):
    global LAST_RESULTS
    from concourse.bass_utils import run_bass_kernel_spmd

    mode = os.environ.get("MOE_MM_DTYPE", "f32r")
    nc = _get_nc(mode)

    xf = np.ascontiguousarray(np.asarray(x, dtype=np.float32).reshape(T, D))
    up = np.asarray(up_w, dtype=np.float32)
    dn = np.asarray(down_w, dtype=np.float32)

    xts = [np.ascontiguousarray(xf[tg * TC:(tg + 1) * TC, :].T) for tg in range(TG)]
    upts = [np.ascontiguousarray(up[eg * EC:(eg + 1) * EC, :].T) for eg in range(EG)]
    dnts = [np.ascontiguousarray(dn[:, eg * EC:(eg + 1) * EC].T) for eg in range(EG)]

    in_maps = []
    for c in range(8):
        tg, eg = c // EG, c % EG
        in_maps.append({"xt": xts[tg], "upw": upts[eg], "dwn": dnts[eg]})

    res = run_bass_kernel_spmd(nc, in_maps, list(range(8)))
    LAST_RESULTS = res

    out = np.empty((T, D), dtype=np.float32)
    for tg in range(TG):
        part = res.results[tg * EG]["ytp"] + res.results[tg * EG + 1]["ytp"]
        out[tg * TC:(tg + 1) * TC, :] = part.T
    return out.reshape(B, S, D)

